# revision 38
# baseline (speedup 1.0000x reference)
"""3-layer GAT (gnn_message_passing) on 8 Trainium2 NeuronCores.

v2 strategy (nodes sharded by dst octant, 6250/core):
  - matmul phase computes h plus u=exp(es), u'=exp(0.2*es) per node
    (table row [h | u | u']), and v=exp(ed), v'=exp(0.2*ed) kept in SBUF.
    Uses exp(lrelu(es+ed)) == max(u*v, u'*v') (exp is monotonic) so the
    per-edge softmax weight is separable: no per-edge ed gather at all.
  - table AllGathered in two halves (row space remapped so each half is
    contiguous); A-group gather indices only need half 0, B-group only
    half 1, so gathers start as soon as their half lands.
  - per dst-tile aggregation: dma_gather rows by src id (the only SWDGE
    descriptor stream left), masks generated on DVE in bf16 in BOTH
    orientations (slot-major msk for the aggregation matmuls, dst-major
    mskT for the per-chunk v-lookup matmul), numerator+denominator
    accumulate across the whole pair in PSUM, epilogue adds the
    self-loop term (core-local rows streamed contiguously - no gather),
    normalizes, bias(+relu), transposes into the next layer's input.
  - next-layer matmuls interleave into the aggregation pair loop so the
    AllGather overlaps the previous layer's tail.
"""
import os
import sys

sys.path.insert(0, "/opt/trn_rl_repo")

import numpy as np
import ml_dtypes

BF16NP = ml_dtypes.bfloat16

MAX_WAITS = 1


def _split_multiwait(nc):
    """walrus in this env rejects >1 sync-wait per instruction: split excess
    waits onto same-engine NoOps."""
    import concourse.mybir as mybir
    for _name, bbb in nc.bb_map.items():
        il = bbb.bb.instructions
        new = []
        changed = False
        for inst in il:
            si = getattr(inst, "sync_info", None)
            ow = list(si.on_wait) if si is not None and si.on_wait else []
            if len(ow) > MAX_WAITS:
                excess, keep = ow[:-MAX_WAITS], ow[-MAX_WAITS:]
                for j, w in enumerate(excess):
                    new.append(mybir.InstNoOp(
                        name=f"{inst.name}_sw{j}",
                        engine=inst.engine,
                        bass_nofuse=True,
                        sync_info=mybir.SyncInfo(on_wait=[w], on_update=[]),
                    ))
                inst.sync_info = mybir.SyncInfo(
                    on_wait=keep, on_update=list(si.on_update))
                changed = True
            new.append(inst)
        if changed:
            bbb.bb.instructions = new


# --------------------------------------------------------------------------
# host-side edge preprocessing
# --------------------------------------------------------------------------
def wrap16_block(vals, num):
    """dma_gather index layout: element i -> [i%16, i//16]; [16, num//16]."""
    a = np.zeros((16, num // 16), dtype=np.int16)
    v = np.asarray(vals, dtype=np.int64)
    i = np.arange(len(v))
    a[i % 16, i // 16] = v.astype(np.int16)
    return a


def splits(cfg):
    """Per-core row-count of each AllGather split (3 splits, last smallest
    so its tail AG is cheap to wait on). Split 0 must equal ATH-1 rows
    globally (the int16 A-group)."""
    R = cfg["R"]
    s0 = R // 2
    s2 = R // 5
    s1 = R - s0 - s2
    return [s0, s1, s2]


def table_row(n, cfg):
    """Global node id -> table row id (1-based, splits contiguous)."""
    R, P = cfg["R"], cfg["P"]
    SPL = splits(cfg)
    OFF = np.cumsum([0] + SPL)[:3]
    CUM = np.cumsum([0] + [P * s for s in SPL])[:3]
    n = np.asarray(n, np.int64)
    core = n // R
    local = n % R
    s = np.searchsorted(np.array([OFF[1], OFF[2]]), local, side="right")
    SPLa = np.array(SPL)
    return 1 + CUM[s] + core * SPLa[s] + (local - OFF[s])


def build_edges(edge_index, cfg):
    """Per-core gather index/metadata arrays.  Self-loops NOT included
    (handled locally); natural (i,i) edges in edge_index stay.

    Slot layout per pair g=(2g,2g+1): [t0A][t1A][t0B][t1B], each
    (tile,group) segment 128-aligned.  A = table rows < ATH (half 0),
    B = rows >= ATH (half 1).  Chunk counts shared across cores (SPMD).
    """
    N, P, R, TPC, ATH = cfg["N"], cfg["P"], cfg["R"], cfg["TPC"], cfg["ATH"]
    src = edge_index[0].astype(np.int64)
    dst = edge_index[1].astype(np.int64)
    owner = dst // R

    cores = []
    cntA = np.zeros((P, TPC), np.int64)
    cntB = np.zeros((P, TPC), np.int64)
    for p in range(P):
        sel = np.nonzero(owner == p)[0]
        d = dst[sel] - p * R
        row = table_row(src[sel], cfg)
        grp = (row >= ATH).astype(np.int64)
        order = np.lexsort((grp, d))
        d = d[order]; row = row[order]; grp = grp[order]
        t = d // 128
        for g in (0, 1):
            c = np.bincount(t[grp == g], minlength=TPC)
            (cntA if g == 0 else cntB)[p] = c
        cores.append((d, row, grp, t))

    nA = np.maximum((cntA.max(axis=0) + 127) // 128, 1)
    nB = np.maximum((cntB.max(axis=0) + 127) // 128, 1)

    npairs = (TPC + 1) // 2
    pair_meta = []  # (tiles, aseg, bseg, chunk0); +len(tiles) self chunks
    chunk0 = 0
    for g in range(npairs):
        tiles = [2 * g] + ([2 * g + 1] if 2 * g + 1 < TPC else [])
        a = [int(nA[t]) for t in tiles]
        b = [int(nB[t]) for t in tiles]
        pair_meta.append((tiles, a, b, chunk0))
        chunk0 += sum(a) + sum(b) + len(tiles)
    NCH = chunk0

    out = []
    for p in range(P):
        d, row, grp, t = cores[p]
        idxA_cols = []
        idxB_cols = []
        dstrow = np.zeros((128, NCH), np.float32)
        c0 = 0
        for (tiles, a, b, _c0) in pair_meta:
            va_parts = []   # A slot rows (table row ids), in slot order
            vb_parts = []
            r_parts_a = []  # tile-local dst row per slot
            r_parts_b = []
            for ti, tt in enumerate(tiles):
                selA = np.nonzero((t == tt) & (grp == 0))[0]
                ns = a[ti] * 128
                va = np.zeros(ns, np.int64)          # pad -> dummy row 0
                va[:len(selA)] = row[selA]
                ra = np.zeros(ns, np.int64)
                ra[:len(selA)] = d[selA] - tt * 128
                va_parts.append(va); r_parts_a.append(ra)
            for ti, tt in enumerate(tiles):
                selB = np.nonzero((t == tt) & (grp == 1))[0]
                ns = b[ti] * 128
                vb = np.full(ns, N + 1, np.int64)    # pad -> dummy row N+1
                vb[:len(selB)] = row[selB]
                rb = np.zeros(ns, np.int64)
                rb[:len(selB)] = d[selB] - tt * 128
                vb_parts.append(vb); r_parts_b.append(rb)
            va_all = np.concatenate(va_parts)
            vb_all = np.concatenate(vb_parts) - ATH
            idxA_cols.append(wrap16_block(va_all, len(va_all)))
            idxB_cols.append(wrap16_block(vb_all, len(vb_all)))
            # self chunks: slot p -> dst p (identity mask), one per tile
            r_parts_s = [np.arange(128, dtype=np.int64) for _ in tiles]
            r_all = np.concatenate(r_parts_a + r_parts_b + r_parts_s)
            ncp = len(r_all) // 128
            dstrow[:, c0:c0 + ncp] = r_all.reshape(ncp, 128).T
            c0 += ncp
        idxA = np.tile(np.concatenate(idxA_cols, axis=1), (8, 1))
        idxB = np.tile(np.concatenate(idxB_cols, axis=1), (8, 1))
        # dst-major replica of dstrow: [128(part, replicated), NCH*128]
        dstrowT = np.ascontiguousarray(
            np.broadcast_to(dstrow.T.reshape(1, NCH * 128), (128, NCH * 128)))
        out.append({"idxA": idxA, "idxB": idxB,
                    "dstrow": dstrow.astype(BF16NP),
                    "dstrowT": dstrowT.astype(BF16NP)})
    return out, pair_meta, NCH


# --------------------------------------------------------------------------
# device program
# --------------------------------------------------------------------------
def build_program(cfg, pair_meta, NCH):
    import concourse.bass as bass
    import concourse.mybir as mybir
    import concourse.tile as tile
    from concourse.library_config import mlp
    from concourse.masks import make_identity
    from concourse.tile_rust import add_dep_helper

    def _mi(x):
        return getattr(x, "ins", x)

    def dep(a, b, why):
        add_dep_helper(_mi(a), _mi(b), reason=why)

    F32 = mybir.dt.float32
    BF16 = mybir.dt.bfloat16
    I16 = mybir.dt.int16
    AF = mybir.ActivationFunctionType
    OP = mybir.AluOpType

    N, P, R, TPC = cfg["N"], cfg["P"], cfg["R"], cfg["TPC"]
    SPL = splits(cfg)
    OFF = [0, SPL[0], SPL[0] + SPL[1]]
    CUM = [0, P * SPL[0], P * (SPL[0] + SPL[1])]
    F_IN, HID, HEADS, OUT = cfg["F_IN"], cfg["HID"], cfg["HEADS"], cfg["OUT"]
    ATH = cfg["ATH"]
    HC = HID * HEADS
    IN2 = HC + F_IN
    TROW = cfg["TROW"]            # 640 bf16: [h 512 | u 8 | u' 8 | pad]
    TROW3 = cfg["TROW3"]          # 128 bf16: [h 16 | u 1 | u' 1 | pad]
    GROW = HC + 2 * HEADS         # 528: gathered bytes per row (elem_size)
    GROW3 = OUT + 2               # 18
    NRT = N + 2
    NTILE = TPC * 128
    MPS = 8                       # chunks per compute slice
    SUBG = 8                      # max chunks per gather call (129 descs/ring
                                  # at 16 hangs HW: ring carveout is 128)

    CA = sum(m[1][i] for m in pair_meta for i in range(len(m[0])))
    CB = sum(m[2][i] for m in pair_meta for i in range(len(m[0])))

    nc = bass.Bass(num_swdge_queues=2)

    ps = {}
    def par(name, shape, dt):
        ps[name] = nc.declare_dram_parameter(name, list(shape), dt,
                                             isOutput=False)
        return ps[name]

    xT = par("xT", [F_IN, R], BF16)
    Wm1 = par("Wm1", [F_IN, HC], BF16)
    Wa1 = par("Wa1", [F_IN, 2 * HEADS], BF16)
    Wm2 = par("Wm2", [IN2, HC], BF16)
    Wa2 = par("Wa2", [IN2, 2 * HEADS], BF16)
    Wm3 = par("Wm3", [IN2, OUT], BF16)
    Wa3 = par("Wa3", [IN2, 2], BF16)
    b1 = par("b1", [128, HC], F32)
    b2 = par("b2", [128, HC], F32)
    b3 = par("b3", [128, OUT], F32)
    idxA_p = par("idxA", [128, CA * 8], I16)
    idxB_p = par("idxB", [128, CB * 8], I16)
    dstrow_p = par("dstrow", [128, NCH], BF16)
    dstrowT_p = par("dstrowT", [128, NCH * 128], BF16)
    dumT = par("dumT", [1, TROW], BF16)     # zeros (u=u'=0 kills pads)
    dumT3 = par("dumT3", [1, TROW3], BF16)
    iota_p = par("iota", [128, 128], BF16)  # value = free index
    iotaP_p = par("iotaP", [128, 1], BF16)  # value = partition index
    out_ext = nc.declare_dram_parameter("out", [R, OUT], F32, isOutput=True)

    # internal DRAM
    T1 = nc.dram_tensor("T1", [NRT, TROW], BF16, addr_space="Shared")
    T2 = nc.dram_tensor("T2", [NRT, TROW], BF16, addr_space="Shared")
    T3 = nc.dram_tensor("T3", [NRT, TROW3], BF16, addr_space="Shared")
    Tsh = {}
    for l, tr in ((1, TROW), (2, TROW), (3, TROW3)):
        for s in range(3):
            Tsh[(l, s)] = nc.dram_tensor(f"Tsh{l}{s}", [SPL[s], tr], BF16)
    Tl = {1: T1, 2: T2, 3: T3}

    nc.gpsimd.load_library(mlp)

    from concourse import ap_utils

    def dma_gather_raw(out_ap, in_ap, idxs_ap, num_idxs, num_idxs_reg,
                       elem_size, elem_step, queue_num=0):
        """bass dma_gather minus the elem_size%256 assert: gather elem_size
        elements from rows strided elem_step apart (both in elements)."""
        g = nc.gpsimd
        assert idxs_ap.dtype == mybir.dt.int16
        assert in_ap.dtype == out_ap.dtype
        assert ap_utils.ap_is_contiguous(out_ap.ap[1:])
        assert ap_utils.ap_is_contiguous(idxs_ap.ap[1:])
        assert in_ap.ap[-1][1] == out_ap.ap[-1][1] == elem_size
        assert in_ap.ap[0][0] == elem_step
        stride_bytes = elem_step * mybir.dt.size(in_ap.dtype)
        assert stride_bytes % 256 == 0 and stride_bytes // 256 < 256
        _in_ap = g.lower_ap_dma(in_ap, for_custom_bir_dma=True)
        return g.add_instruction(
            mybir.InstDMAGatherAnt(
                name=nc.get_next_instruction_name(),
                ins=[*_in_ap, g.lower_ap(idxs_ap),
                     g.lower_val_access(g.to_reg(num_idxs_reg))],
                outs=[g.lower_ap(out_ap)],
                transpose=False,
                num_idxs=num_idxs,
                elem_size=elem_size,
                stride_bytes_256=stride_bytes // 256,
                gen_mode=0,
                single_packet=True,
                queue_num=queue_num,
                sbuf_tokens_per_rank=0,
                sbuf_free_dim_per_rank=0,
                sbuf_free_dim_pad_per_rank=0,
                sbuf_byte_offset=0,
            ))

    from contextlib import ExitStack
    _regstack = ExitStack()
    _regcache = {}

    def numreg(v):
        if v not in _regcache:
            r = _regstack.enter_context(nc.gpsimd.register(f"nidx{v}"))
            nc.gpsimd.reg_mov(r, v)
            _regcache[v] = r
        return _regcache[v]

    def lcfg(layer):
        if layer == 3:
            return dict(nh=1, hcols=OUT, trow=TROW3, grow=GROW3, T_=T3,
                        nfc=IN2 // 128)
        return dict(nh=HEADS, hcols=HC, trow=TROW, grow=GROW,
                    T_=T1 if layer == 1 else T2,
                    nfc=(F_IN if layer == 1 else IN2) // 128)

    # tile -> list of (split, src_row0, n, dst_part0) covering its rows
    def tile_spans(t):
        r0 = t * 128
        vr = min(128, R - r0)
        spans = []
        p0 = 0
        while p0 < vr:
            lr = r0 + p0
            s = 2 if lr >= OFF[2] else (1 if lr >= OFF[1] else 0)
            base = lr - OFF[s]
            n = min(vr - p0, OFF[s] + SPL[s] - lr)
            spans.append((s, base, n, p0))
            p0 += n
        return spans, vr

    # last tile whose rows complete split s
    split_last_tile = [(OFF[s] + SPL[s] - 1) // 128 for s in range(3)]

    with tile.TileContext(nc) as tc:
        with (
            tc.tile_pool(name="const", bufs=1) as constp,
            tc.tile_pool(name="w", bufs=1) as wp,
            tc.tile_pool(name="outT", bufs=1) as outTp,
            tc.tile_pool(name="xtl", bufs=2) as xtlp,
            tc.tile_pool(name="zrow", bufs=2) as zrowp,
            tc.tile_pool(name="vv", bufs=2) as vvp,
            tc.tile_pool(name="gAB", bufs=3) as gABp,
            tc.tile_pool(name="drt", bufs=2) as drtp,
            tc.tile_pool(name="msk", bufs=2) as mskp,
            tc.tile_pool(name="mskT", bufs=2) as mskTp,
            tc.tile_pool(name="seg", bufs=2) as segp,
            tc.tile_pool(name="selfr", bufs=2) as selfp,
            tc.tile_pool(name="ep", bufs=2) as epp,
            tc.tile_pool(name="psA", bufs=2, space="PSUM") as psAp,
            tc.tile_pool(name="psDen", bufs=2, space="PSUM") as psDp,
            tc.tile_pool(name="psX", bufs=2, space="PSUM") as psXp,
            tc.tile_pool(name="psM", bufs=1, space="PSUM") as psMp,
            tc.tile_pool(name="psW", bufs=1, space="PSUM") as psWp,
        ):
            # ---------- constants / resident data
            ident = constp.tile([128, 128], BF16, tag="ident")
            make_identity(nc, ident[:])
            iota_sb = constp.tile([128, 128], BF16, tag="iota")
            nc.sync.dma_start(out=iota_sb[:], in_=iota_p[:])
            iotaP_sb = constp.tile([128, 1], BF16, tag="iotaP")
            nc.sync.dma_start(out=iotaP_sb[:], in_=iotaP_p[:])

            idxA_sb = constp.tile([128, CA * 8], I16, tag="idxA")
            nc.sync.dma_start(out=idxA_sb[:], in_=idxA_p[:])
            idxB_sb = constp.tile([128, CB * 8], I16, tag="idxB")
            nc.sync.dma_start(out=idxB_sb[:], in_=idxB_p[:])
            dstrow_sb = constp.tile([128, NCH], BF16, tag="dstrow")
            nc.sync.dma_start(out=dstrow_sb[:], in_=dstrow_p[:])

            bias_sb = {}
            for nm, p_, w_ in (("b1", b1, HC), ("b2", b2, HC), ("b3", b3, OUT)):
                bias_sb[nm] = constp.tile([128, w_], F32, tag=nm, name=nm)
                nc.sync.dma_start(out=bias_sb[nm][:], in_=p_[:])

            dummy_w = {}
            for T_, dum in ((T1, dumT), (T2, dumT), (T3, dumT3)):
                i1 = nc.sync.dma_start(out=T_[0:1, :], in_=dum[:])
                i2 = nc.sync.dma_start(out=T_[N + 1:N + 2, :], in_=dum[:])
                dummy_w[id(T_)] = [i1, i2]

            def load_w(p_, rows, cols, tag):
                nchunks = (rows + 127) // 128
                tl = wp.tile([128, nchunks * cols], BF16, tag=tag)
                for fc in range(nchunks):
                    r0 = fc * 128
                    vr = min(128, rows - r0)
                    nc.sync.dma_start(out=tl[:vr, fc * cols:(fc + 1) * cols],
                                      in_=p_[r0:r0 + vr, :])
                return tl

            Wm_sb = {1: load_w(Wm1, F_IN, HC, "Wm1"),
                     2: load_w(Wm2, IN2, HC, "Wm2"),
                     3: load_w(Wm3, IN2, OUT, "Wm3")}
            Wa_sb = {1: load_w(Wa1, F_IN, 2 * HEADS, "Wa1"),
                     2: load_w(Wa2, IN2, 2 * HEADS, "Wa2"),
                     3: load_w(Wa3, IN2, 2, "Wa3")}

            outT_sb = outTp.tile([128, (HC // 128) * NTILE], BF16, tag="outT")

            vv_sb = {}       # layer -> tile [128, TPC*2*nh] bf16
            tsh_writes = {}  # (layer, half) -> [dma insts]
            tsh_tile_w = {}  # (layer, tile) -> [dma insts]
            cc_done = {}     # (layer, half) -> cc inst

            # per-pair gather index column offsets (shared by all layers)
            pair_offs = []
            oa = ob_ = 0
            for (tiles_, aseg_, bseg_, _c0) in pair_meta:
                pair_offs.append((oa, ob_))
                oa += sum(aseg_) * 8
                ob_ += sum(bseg_) * 8

            # ---------- one row-tile of the matmul phase ----------
            def matmul_tile(layer, t):
                L = lcfg(layer)
                nh, hcols, trow, nfc = L["nh"], L["hcols"], L["trow"], L["nfc"]
                nxc = HC // 128
                spans, vr = tile_spans(t)
                r0 = t * 128
                Wm = Wm_sb[layer]
                Wa = Wa_sb[layer]
                acols = 2 * nh

                # stream x columns for this tile (L1: fc 0..1; L2/3: fc 4..5)
                nxf = F_IN // 128
                xtl = xtlp.tile([128, nxf * 128], BF16, tag="xtl")
                nc.sync.dma_start(
                    out=xtl[:, :nxf * vr].rearrange("p (f c) -> p f c", f=nxf),
                    in_=xT[:, r0:r0 + vr].rearrange("(f p) c -> p f c", p=128))

                pm = psMp.tile([128, max(hcols, 8)], F32, tag="pm")
                pa = psWp.tile([128, 16], F32, tag="pa")
                for fc in range(nfc):
                    if layer == 1:
                        lhsT = xtl[:, fc * vr:(fc + 1) * vr]
                    elif fc < nxc:
                        lhsT = outT_sb[:, fc * NTILE + r0:fc * NTILE + r0 + vr]
                    else:
                        fx = fc - nxc
                        lhsT = xtl[:, fx * vr:(fx + 1) * vr]
                    nc.tensor.matmul(out=pm[:vr, :hcols], lhsT=lhsT,
                                     rhs=Wm[:, fc * hcols:(fc + 1) * hcols],
                                     start=(fc == 0), stop=(fc == nfc - 1))
                    nc.tensor.matmul(out=pa[:vr, :acols], lhsT=lhsT,
                                     rhs=Wa[:, fc * acols:(fc + 1) * acols],
                                     start=(fc == 0), stop=(fc == nfc - 1))

                zrow = zrowp.tile([128, trow], BF16, tag="zrow")
                nc.vector.memset(zrow[:, hcols + 2 * nh:], 0.0)
                if vr < 128:
                    nc.vector.memset(zrow[:, :hcols + 2 * nh], 0.0)
                nc.scalar.copy(out=zrow[:vr, :hcols], in_=pm[:vr, :hcols])
                nc.scalar.activation(out=zrow[:vr, hcols:hcols + nh],
                                     in_=pa[:vr, 0:nh], func=AF.Exp)
                nc.scalar.activation(out=zrow[:vr, hcols + nh:hcols + 2 * nh],
                                     in_=pa[:vr, 0:nh], func=AF.Exp, scale=0.2)
                vv = vv_sb[layer]
                if vr < 128:
                    # garbage rows would propagate NaN through the v-lookup
                    # matmul (0 * inf); zero the whole tile block first
                    nc.vector.memset(vv[:, t * 2 * nh:(t + 1) * 2 * nh], 0.0)
                nc.scalar.activation(out=vv[:vr, t * 2 * nh:t * 2 * nh + nh],
                                     in_=pa[:vr, nh:2 * nh], func=AF.Exp)
                nc.scalar.activation(out=vv[:vr, t * 2 * nh + nh:(t + 1) * 2 * nh],
                                     in_=pa[:vr, nh:2 * nh], func=AF.Exp,
                                     scale=0.2)

                for (half, base, n, p0) in spans:
                    iw = nc.sync.dma_start(
                        out=Tsh[(layer, half)][base:base + n, :],
                        in_=zrow[p0:p0 + n, :])
                    tsh_writes.setdefault((layer, half), []).append(iw)
                    tsh_tile_w.setdefault((layer, t), []).append(iw)

            def issue_ag(layer, s):
                T_ = Tl[layer]
                o0 = 1 + CUM[s]
                cc = nc.gpsimd.collective_compute(
                    "AllGather",
                    mybir.AluOpType.bypass,
                    replica_groups=[list(range(P))],
                    ins=[Tsh[(layer, s)][:, :]],
                    outs=[T_[o0:o0 + P * SPL[s], :]],
                )
                for w_ in tsh_writes.get((layer, s), []):
                    dep(cc, w_, "allgather reads shard writes")
                cc_done[(layer, s)] = cc

            # ---------- aggregation ----------
            _gq = [0]  # alternating SWDGE queue
            _pair_state = {}  # gidx -> (bufAB, meta...)

            def pair_calc(gidx):
                tiles, aseg, bseg, c0 = pair_meta[gidx]
                na, nb = sum(aseg), sum(bseg)
                ns = len(tiles)
                ncp = na + nb + ns
                ctile = []
                for ti in range(ns):
                    ctile += [ti] * aseg[ti]
                for ti in range(ns):
                    ctile += [ti] * bseg[ti]
                ctile += list(range(ns))
                return tiles, c0, na, nb, ns, ncp, ctile

            def gather_pair(layer, gidx, part):
                """Emit the Pool gather calls for one pair (part in A/B);
                part B also emits the HWDGE self-row fills."""
                L = lcfg(layer)
                trow, T_, grow = L["trow"], L["T_"], L["grow"]
                tiles, c0, na, nb, ns, ncp, ctile = pair_calc(gidx)

                if part == "A":
                    bufAB = gABp.tile([128, ncp * grow], BF16, tag="bufAB",
                                      name=f"bufAB{gidx}")
                    _pair_state[gidx] = bufAB
                    n0, cnt, off, idx_sb = 0, na, pair_offs[gidx][0], idxA_sb
                    base_ap = T_[:, 0:grow]
                else:
                    bufAB = _pair_state[gidx]
                    n0, cnt, off, idx_sb = na, nb, pair_offs[gidx][1], idxB_sb
                    base_ap = T_[ATH:, 0:grow]

                for cs in range(0, cnt, SUBG):
                    ck = min(SUBG, cnt - cs)
                    g_ = dma_gather_raw(
                        bufAB[:, (n0 + cs) * grow:(n0 + cs + ck) * grow]
                        .rearrange("p (c w) -> p c w", w=grow),
                        base_ap,
                        idx_sb[:, off + cs * 8:off + (cs + ck) * 8],
                        ck * 128, numreg(ck * 128), grow, trow,
                        queue_num=_gq[0])
                    _gq[0] ^= 1
                    if part == "A":
                        dep(g_, cc_done[(layer, 0)], "A reads split 0")
                    else:
                        dep(g_, cc_done[(layer, 1)], "B reads split 1")
                        dep(g_, cc_done[(layer, 2)], "B reads split 2")
                    for d_ in dummy_w[id(T_)]:
                        dep(g_, d_, "gather reads dummy rows")

                if part == "B":
                    # self chunks: core-local rows streamed from Tsh
                    for ti, t in enumerate(tiles):
                        cS = na + nb + ti
                        spans, vr = tile_spans(t)
                        if vr < 128:
                            nc.vector.memset(
                                bufAB[:, cS * grow:(cS + 1) * grow], 0.0)
                        for (sp, base, n, p0) in spans:
                            iw = nc.sync.dma_start(
                                out=bufAB[p0:p0 + n,
                                          cS * grow:(cS + 1) * grow],
                                in_=Tsh[(layer, sp)][base:base + n, 0:grow])
                            for w_ in tsh_tile_w.get((layer, t), []):
                                dep(iw, w_, "self rows read shard")

            def agg_pair(layer, gidx, do_next_matmul):
                L = lcfg(layer)
                nh, hcols, trow, T_ = L["nh"], L["hcols"], L["trow"], L["T_"]
                grow = L["grow"]
                tiles, c0, na, nb, ns, ncp, ctile = pair_calc(gidx)
                vv = vv_sb[layer]
                bufAB = _pair_state.pop(gidx)
                first_c = {}
                last_c = {}
                for j, ti in enumerate(ctile):
                    first_c.setdefault(ti, j)
                    last_c[ti] = j

                drt = drtp.tile([128, ncp * 128], BF16, tag="drt")
                nc.sync.dma_start(
                    out=drt[:],
                    in_=dstrowT_p[:, c0 * 128:(c0 + ncp) * 128])

                pag = [psAp.tile([128, max(hcols, 8)], F32, tag="pag",
                                 name=f"pag{ti}") for ti in range(len(tiles))]
                pde = [psDp.tile([128, 8], F32, tag="pde",
                                 name=f"pde{ti}") for ti in range(len(tiles))]

                bv = bufAB[:].rearrange("p (c w) -> p c w", w=grow)
                for cs in range(0, ncp, MPS):
                    ck = min(MPS, ncp - cs)
                    mskT = mskTp.tile([128, MPS * 128], BF16, tag="mskT")
                    nc.vector.tensor_tensor(
                        out=mskT[:, :ck * 128].rearrange(
                            "p (c s) -> p c s", s=128),
                        in0=iotaP_sb[:, :, None].to_broadcast([128, ck, 128]),
                        in1=drt[:, cs * 128:(cs + ck) * 128].rearrange(
                            "p (c s) -> p c s", s=128),
                        op=OP.is_equal)
                    msk = mskp.tile([128, MPS * 128], BF16, tag="msk")
                    nc.vector.tensor_tensor(
                        out=msk[:, :ck * 128].rearrange(
                            "p (c r) -> p c r", r=128),
                        in0=dstrow_sb[:, c0 + cs:c0 + cs + ck, None]
                        .to_broadcast([128, ck, 128]),
                        in1=iota_sb[:, None, :].to_broadcast([128, ck, 128]),
                        op=OP.is_equal)
                    # v-lookup per chunk
                    pvv = psXp.tile([128, MPS * 2 * nh], F32, tag="pvv",
                                    name="pvv", bufs=1)
                    for j in range(ck):
                        tl = ctile[cs + j]
                        nc.tensor.matmul(
                            out=pvv[:, j * 2 * nh:(j + 1) * 2 * nh],
                            lhsT=mskT[:, j * 128:(j + 1) * 128],
                            rhs=vv[:, (tiles[tl] * 2 * nh):
                                   (tiles[tl] + 1) * 2 * nh],
                            start=True, stop=True, skip_group_check=True)
                    vex = segp.tile([128, MPS * 2 * nh], BF16, tag="vex")
                    nc.scalar.copy(out=vex[:, :ck * 2 * nh],
                                   in_=pvv[:, :ck * 2 * nh])
                    vex3 = vex[:, :ck * 2 * nh].rearrange(
                        "p (c v) -> p c v", v=2 * nh)
                    t1 = segp.tile([128, MPS * nh], BF16, tag="t1")
                    t2 = segp.tile([128, MPS * nh], BF16, tag="t2")
                    ex = segp.tile([128, MPS * nh], BF16, tag="ex")
                    nc.vector.tensor_tensor(
                        out=t1[:, :ck * nh].rearrange("p (c h) -> p c h", h=nh),
                        in0=bv[:, cs:cs + ck, hcols:hcols + nh],
                        in1=vex3[:, :, 0:nh], op=OP.mult)
                    nc.vector.tensor_tensor(
                        out=t2[:, :ck * nh].rearrange("p (c h) -> p c h", h=nh),
                        in0=bv[:, cs:cs + ck, hcols + nh:hcols + 2 * nh],
                        in1=vex3[:, :, nh:2 * nh], op=OP.mult)
                    nc.vector.tensor_tensor(
                        out=ex[:, :ck * nh], in0=t1[:, :ck * nh],
                        in1=t2[:, :ck * nh], op=OP.max)
                    mp_ = segp.tile([128, MPS * hcols], BF16, tag="mp")
                    nc.vector.tensor_tensor(
                        out=mp_[:, :ck * hcols].rearrange(
                            "p (c h k) -> p c h k", c=ck, h=nh),
                        in0=bv[:, cs:cs + ck, 0:hcols].rearrange(
                            "p c (h k) -> p c h k", h=nh),
                        in1=ex[:, :ck * nh].rearrange(
                            "p (c h) -> p c h", h=nh)[:, :, :, None]
                        .to_broadcast([128, ck, nh, hcols // nh]),
                        op=OP.mult)
                    for j in range(ck):
                        tl = ctile[cs + j]
                        cj = cs + j
                        nc.tensor.matmul(
                            out=pag[tl][:, :hcols],
                            lhsT=msk[:, j * 128:(j + 1) * 128],
                            rhs=mp_[:, j * hcols:(j + 1) * hcols],
                            start=(cj == first_c[tl]), stop=(cj == last_c[tl]),
                            skip_group_check=True)
                        nc.tensor.matmul(
                            out=pde[tl][:, :nh],
                            lhsT=msk[:, j * 128:(j + 1) * 128],
                            rhs=ex[:, j * nh:(j + 1) * nh],
                            start=(cj == first_c[tl]), stop=(cj == last_c[tl]),
                            skip_group_check=True)

                # ---------- epilogue per tile
                for ti, t in enumerate(tiles):
                    spans, vr = tile_spans(t)
                    r0 = t * 128
                    den = epp.tile([128, 8], F32, tag="den")
                    nc.vector.reciprocal(out=den[:, :nh], in_=pde[ti][:, :nh])
                    o1 = epp.tile([128, max(hcols, 8)], F32, tag="o1")
                    nc.vector.tensor_tensor(
                        out=o1[:, :hcols].rearrange("p (h k) -> p h k", h=nh),
                        in0=pag[ti][:, :hcols].rearrange(
                            "p (h k) -> p h k", h=nh),
                        in1=den[:, :nh, None].to_broadcast(
                            [128, nh, hcols // nh]),
                        op=OP.mult)
                    bias = bias_sb[f"b{layer}"]
                    nc.vector.tensor_tensor(
                        out=o1[:, :hcols], in0=o1[:, :hcols],
                        in1=bias[:, :], op=OP.add)
                    if layer != 3:
                        ob = epp.tile([128, hcols], BF16, tag="ob")
                        nc.scalar.activation(out=ob[:, :], in_=o1[:, :hcols],
                                             func=AF.Relu)
                        for q in range(hcols // 128):
                            pt = psXp.tile([128, 128], BF16, tag="pt",
                                           name="pt", bufs=1)
                            nc.tensor.transpose(
                                out=pt[:, :vr],
                                in_=ob[:vr, q * 128:(q + 1) * 128],
                                identity=ident[:vr, :vr])
                            nc.scalar.copy(
                                out=outT_sb[:, q * NTILE + r0:
                                            q * NTILE + r0 + vr],
                                in_=pt[:, :vr])
                    else:
                        # log-softmax without max-subtraction: o1 is small
                        # enough that exp stays in f32 range
                        ex3 = epp.tile([128, hcols], F32, tag="ex3")
                        s3 = epp.tile([128, 1], F32, tag="s3")
                        nc.scalar.activation(out=ex3[:], in_=o1[:, :hcols],
                                             func=AF.Exp, accum_out=s3[:])
                        ln3 = epp.tile([128, 1], F32, tag="ln3")
                        nc.scalar.activation(out=ln3[:], in_=s3[:], func=AF.Ln)
                        res = epp.tile([128, hcols], F32, tag="res")
                        nc.vector.tensor_scalar(
                            out=res[:], in0=o1[:, :hcols], scalar1=ln3[:],
                            scalar2=None, op0=OP.subtract)
                        nc.sync.dma_start(out=out_ext[r0:r0 + vr, :],
                                          in_=res[:vr, :])

                    if do_next_matmul:
                        matmul_tile(layer + 1, t)

            # ---------- program ----------
            vv_sb[1] = vvp.tile([128, TPC * 2 * HEADS], BF16, tag="vv",
                                name="vv1")
            for t in range(TPC):
                matmul_tile(1, t)
                for s in range(3):
                    if t == split_last_tile[s]:
                        issue_ag(1, s)

            # cc triggers are cheap doorbell writes; fire each split's AG as
            # soon as its shard rows exist (one pair after the last tile, so
            # the Pool wait matches the bufAB WAR throttle anyway)
            cc_pairs = {}
            late_splits = []
            for s in range(3):
                gp = split_last_tile[s] // 2 + 1
                if gp <= len(pair_meta) - 1:
                    cc_pairs.setdefault(gp, []).append(s)
                else:
                    late_splits.append(s)

            def run_layer(layer):
                if layer < 3:
                    nxt_nh = HEADS if layer + 1 < 3 else 1
                    vv_sb[layer + 1] = vvp.tile(
                        [128, TPC * 2 * nxt_nh], BF16, tag="vv",
                        name=f"vv{layer + 1}")
                npair = len(pair_meta)
                # Pool order: A(0) A(1) B(0) B(1), then per g:
                # A(g+2) B(g+2) compute(g) — gathers run 2 pairs ahead so
                # the AG split waits overlap useful emission.
                gather_pair(layer, 0, "A")
                if npair > 1:
                    gather_pair(layer, 1, "A")
                gather_pair(layer, 0, "B")
                if npair > 1:
                    gather_pair(layer, 1, "B")
                for g in range(npair):
                    if g + 2 < npair:
                        gather_pair(layer, g + 2, "A")
                        gather_pair(layer, g + 2, "B")
                    agg_pair(layer, g, do_next_matmul=(layer < 3))
                    if layer < 3:
                        for s in cc_pairs.get(g, []):
                            issue_ag(layer + 1, s)
                if layer < 3:
                    for s in late_splits:
                        issue_ag(layer + 1, s)

            for layer in (1, 2, 3):
                run_layer(layer)

    _regstack.close()
    from concourse.library_overlay import lower_extended_insts
    lower_extended_insts(nc)
    return nc


# --------------------------------------------------------------------------
# host wrapper
# --------------------------------------------------------------------------
def _prep_inputs(inputs, cfg):
    N, P, R, TPC = cfg["N"], cfg["P"], cfg["R"], cfg["TPC"]
    HEADS, HID, OUT, F_IN = cfg["HEADS"], cfg["HID"], cfg["OUT"], cfg["F_IN"]
    HC = HEADS * HID

    x = np.asarray(inputs["x"], np.float32)
    edge_index = np.asarray(inputs["edge_index"], np.int64)

    shards, pair_meta, NCH = build_edges(edge_index, cfg)

    def fold(W, a_s, a_d, heads, ch):
        F = W.shape[0]
        Wr = W.reshape(F, heads, ch)
        Wa = np.zeros((F, 2 * heads), np.float32)
        for h in range(heads):
            Wa[:, h] = Wr[:, h] @ a_s[h]
            Wa[:, heads + h] = Wr[:, h] @ a_d[h]
        return Wa

    w1 = np.asarray(inputs["w1"], np.float32)
    w2 = np.asarray(inputs["w2"], np.float32)
    w3 = np.asarray(inputs["w3"], np.float32)
    Wa1 = fold(w1, np.asarray(inputs["a1s"]), np.asarray(inputs["a1d"]),
               HEADS, HID)
    Wa2 = fold(w2, np.asarray(inputs["a2s"]), np.asarray(inputs["a2d"]),
               HEADS, HID)
    Wa3 = fold(w3, np.asarray(inputs["a3s"]), np.asarray(inputs["a3d"]),
               1, OUT)

    common = {
        "Wm1": w1.astype(BF16NP), "Wa1": Wa1.astype(BF16NP),
        "Wm2": w2.astype(BF16NP), "Wa2": Wa2.astype(BF16NP),
        "Wm3": w3.astype(BF16NP), "Wa3": Wa3.astype(BF16NP),
        "b1": np.tile(np.asarray(inputs["b1"], np.float32).reshape(1, HC),
                      (128, 1)),
        "b2": np.tile(np.asarray(inputs["b2"], np.float32).reshape(1, HC),
                      (128, 1)),
        "b3": np.tile(np.asarray(inputs["b3"], np.float32).reshape(1, OUT),
                      (128, 1)),
        "dumT": np.zeros((1, cfg["TROW"]), BF16NP),
        "dumT3": np.zeros((1, cfg["TROW3"]), BF16NP),
        "iota": np.tile(np.arange(128, dtype=np.float32), (128, 1))
        .astype(BF16NP),
        "iotaP": np.arange(128, dtype=np.float32).reshape(128, 1)
        .astype(BF16NP),
    }
    in_maps = []
    for p in range(P):
        m = dict(common)
        m["xT"] = np.ascontiguousarray(
            x[p * R:(p + 1) * R, :].T).astype(BF16NP)
        m["idxA"] = shards[p]["idxA"]
        m["idxB"] = shards[p]["idxB"]
        m["dstrow"] = shards[p]["dstrow"]
        m["dstrowT"] = shards[p]["dstrowT"]
        in_maps.append(m)
    return in_maps, pair_meta, NCH


def default_cfg():
    return dict(N=50000, P=8, R=6250, TPC=49, F_IN=256, HID=64, HEADS=8,
                OUT=16, ATH=25001, TROW=640, TROW3=128)


def kernel(**inputs):
    cfg = default_cfg()
    in_maps, pair_meta, NCH = _prep_inputs(inputs, cfg)
    nc = build_program(cfg, pair_meta, NCH)

    _split_multiwait(nc)
    from concourse.bass_utils import run_bass_kernel_spmd
    trace = bool(os.environ.get("GNN_TRACE"))
    if trace:
        sys.path.insert(0, "/root/problem/work")
        import axonhook  # noqa
    res = run_bass_kernel_spmd(nc, in_maps, list(range(cfg["P"])),
                               trace=trace)
    if trace:
        kernel.last_exec_ns = res.exec_time_ns
    out = np.concatenate([res.results[p]["out"] for p in range(cfg["P"])],
                         axis=0)
    return out.astype(np.float32)
